# revision 22
# baseline (speedup 1.0000x reference)
"""Trainium2 Bass kernel for per-pixel dot-product attention.

Reference op (per pixel, over C=80 channels split q/k/v = 8/64/8):
    qk[v] = sum_k q[k] * K[k, v] / sqrt(8)
    attn  = softmax(qk over v)
    out[v] = attn[v] * V[v]

Strategy: pure data-parallel over 8 NeuronCores — core i handles batch
i//2, H-rows half (i%2).  Per core all compute is elementwise on
(128, ncol) pixel grids; the 80 channels live as column-blocks of big
SBUF tiles so the whole per-pixel matvec+softmax is ~11 wide vector ops
per chunk (no PSUM / TensorE / transposes).  DVE does the multiplies &
small adds, GPSIMD the big add-tree levels, ScalarE the exp.
"""

import numpy as np

NK = 8
NV = 8
C = NK + NK * NV + NV  # 80
B, H, W = 4, 512, 512
N_CORES = 8
ROWS = H // 2            # rows per core
PIX = ROWS * W           # pixels per core (131072)
NCHUNK = 8               # chunks per core
_SCALE = 1.0 / float(np.sqrt(NK))


def _ensure_path():
    import sys
    p = "/opt/trn_rl_repo"
    if p not in sys.path:
        sys.path.insert(0, p)


def build_nc(pix=PIX, nchunk=NCHUNK, recip_on_act=False, bf16_tree=False,
             k_splits=2, inplace_tree=False, lean_bufs=False, conv_k=False,
             chunk_cols=None, split_rings=False):
    """Build the per-core Bass program for a (80, pix) f32 shard.

    All tensor_tensor work runs on DVE (GPSIMD shares an SBUF port with DVE
    and the two engines serialize, so Pool offload is a net loss).  K streams
    in on the sync HWDGE ring in `k_splits` pieces (compute starts after the
    first piece); q/v loads and the output go on the scalar ring.  With
    `inplace_tree` the l1/l2 add-tree levels write back into the prod tile
    (strictly trailing writes, single-engine serial) to fit ncol=256 in SBUF.
    """
    _ensure_path()
    import concourse.tile as tile
    from concourse import bacc, mybir

    f32 = mybir.dt.float32
    mid = mybir.dt.bfloat16 if bf16_tree else f32
    if chunk_cols is None:
        npix = pix // nchunk
        assert npix % 128 == 0
        chunk_cols = [npix // 128] * nchunk
    assert sum(chunk_cols) * 128 == pix

    nc = bacc.Bacc("TRN2", target_bir_lowering=False, debug=False)
    x = nc.dram_tensor("x", [C, pix], f32, kind="ExternalInput")
    y = nc.dram_tensor("y", [NV, pix], f32, kind="ExternalOutput")

    qv_bufs = 1 if lean_bufs else 2
    pipe_bufs = 1 if lean_bufs else 2
    # deferred output DMAs: emit chunk j's store after chunk j+1's input
    # triggers so it never head-of-line-blocks loads on its ring
    pending_out = []

    def flush_out():
        for args in pending_out:
            nc.scalar.dma_start(**args)
        pending_out.clear()

    with tile.TileContext(nc) as tc:
        with (
            tc.tile_pool(name="inp", bufs=2) as in_pool,
            tc.tile_pool(name="work", bufs=1) as work_pool,
            tc.tile_pool(name="pipe", bufs=pipe_bufs) as pipe_pool,
        ):
            off = 0
            for j, ncol in enumerate(chunk_cols):
                npix = 128 * ncol
                q_t = in_pool.tile([128, NK * ncol], f32, name=f"q{j}", tag="q", bufs=qv_bufs)
                k_t = in_pool.tile([128, NK * NV * ncol], f32, name=f"k{j}", tag="k")
                v_t = in_pool.tile([128, NV * ncol], f32, name=f"v{j}", tag="v", bufs=qv_bufs)

                q_ring = nc.scalar if split_rings else nc.sync
                q_ring.dma_start(
                    out=q_t.rearrange("p (c x) -> p c x", c=NK),
                    in_=x[0:NK, off:off + npix].rearrange("c (p x) -> p c x", p=128),
                )
                # k_t column-block b = k*NV + v holds input channel NK + k*NV + v
                # (k-major matches DRAM channel order → 3D-balanced DMAs);
                # split loads so prod can start early
                k4 = k_t.rearrange("p (k v x) -> p k v x", k=NK, v=NV)
                sp_ch = NK * NV // k_splits
                if conv_k:  # match the B-half-first compute order
                    dma_order = [h for h in range(k_splits) if h * sp_ch >= NK * NV // 2] + \
                                [h for h in range(k_splits) if h * sp_ch < NK * NV // 2]
                else:
                    dma_order = list(range(k_splits))
                for h in dma_order:
                    # descriptor generation is a bottleneck: with split_rings
                    # the B half (needed first) goes on sync, the A half on
                    # scalar, so both HWDGE rings generate in parallel
                    on_sync = (h * sp_ch >= NK * NV // 2) or not split_rings
                    ring = nc.sync if on_sync else nc.scalar
                    ring.dma_start(
                        out=k_t.rearrange("p (c x) -> p c x", c=NK * NV)
                        [:, h * sp_ch:(h + 1) * sp_ch],
                        in_=x[NK + h * sp_ch:NK + (h + 1) * sp_ch, off:off + npix]
                        .rearrange("c (p x) -> p c x", p=128),
                    )
                nc.sync.dma_start(
                    out=v_t.rearrange("p (c x) -> p c x", c=NV),
                    in_=x[NK + NK * NV:C, off:off + npix]
                    .rearrange("c (p x) -> p c x", p=128),
                )
                # previous chunk's output, behind this chunk's input triggers
                flush_out()

                # prod[k,v] = q[k] * K[k,v]   (one broadcast multiply per K piece)
                sp_k = NK // k_splits
                if conv_k:
                    # ScalarE downcasts K into the prod tiles and Q into a small
                    # bf16 tile; DVE then multiplies in-place at bf16 2x rate.
                    # prod is split into two half-tiles: B frees after l1, so
                    # the next chunk's conversions overlap this chunk's tail.
                    assert bf16_tree and inplace_tree
                    half = NK // 2
                    prodA = work_pool.tile([128, half * NV * ncol], mid,
                                           name=f"prodA{j}", tag="prodA")
                    prodB = work_pool.tile([128, half * NV * ncol], mid,
                                           name=f"prodB{j}", tag="prodB")
                    p4A = prodA.rearrange("p (k v x) -> p k v x", k=half, v=NV)
                    p4B = prodB.rearrange("p (k v x) -> p k v x", k=half, v=NV)
                    q_bf = work_pool.tile([128, NK * ncol], mybir.dt.bfloat16,
                                          name=f"qbf{j}", tag="qbf")
                    nc.scalar.activation(q_bf, q_t, mybir.ActivationFunctionType.Copy)
                    q_b = (
                        q_bf.rearrange("p (k x) -> p k x", k=NK)
                        .unsqueeze(2)
                        .broadcast_to((128, NK, NV, ncol))
                    )

                    def pslice(kl, kh):  # view of prod blocks [kl, kh)
                        if kh <= half:
                            return p4A[:, kl:kh]
                        assert kl >= half
                        return p4B[:, kl - half:kh - half]

                    # emit B-half first: its tile frees earliest (after l1)
                    order = [h for h in range(k_splits) if h * sp_k >= half] + \
                            [h for h in range(k_splits) if h * sp_k < half]
                    for h in order:
                        kl, kh = h * sp_k, (h + 1) * sp_k
                        pv = pslice(kl, kh)
                        nc.scalar.activation(pv, k4[:, kl:kh],
                                             mybir.ActivationFunctionType.Copy)
                        nc.vector.tensor_tensor(
                            pv, q_b[:, kl:kh], pv, mybir.AluOpType.mult
                        )
                    # tree: l1 = A + B -> A; l2, qk within A
                    nc.vector.tensor_tensor(p4A, p4A, p4B, mybir.AluOpType.add)
                    l24 = p4A[:, 0:2]
                    nc.vector.tensor_tensor(l24, p4A[:, 0:2], p4A[:, 2:4], mybir.AluOpType.add)
                else:
                    prod = work_pool.tile([128, NK * NV * ncol], mid, name=f"prod{j}", tag="prod")
                    p4 = prod.rearrange("p (k v x) -> p k v x", k=NK, v=NV)
                    q_b = (
                        q_t.rearrange("p (k x) -> p k x", k=NK)
                        .unsqueeze(2)
                        .broadcast_to((128, NK, NV, ncol))
                    )
                    for h in range(k_splits):
                        kl, kh = h * sp_k, (h + 1) * sp_k
                        nc.vector.tensor_tensor(
                            p4[:, kl:kh], q_b[:, kl:kh], k4[:, kl:kh], mybir.AluOpType.mult
                        )

                    # sum over k (outer block index): 3-level pairwise tree (all DVE)
                    if inplace_tree:
                        # l1 -> prod[k 0:4], l2 -> prod[k 0:2]: strictly in-place
                        # (out == in0), serial on DVE
                        l14 = p4[:, 0:4]
                        nc.vector.tensor_tensor(l14, p4[:, 0:4], p4[:, 4:8], mybir.AluOpType.add)
                        l24 = p4[:, 0:2]
                        nc.vector.tensor_tensor(l24, l14[:, 0:2], l14[:, 2:4], mybir.AluOpType.add)
                    else:
                        l1 = work_pool.tile([128, 4 * NV * ncol], mid, name=f"l1_{j}", tag="l1")
                        l14 = l1.rearrange("p (k v x) -> p k v x", k=4, v=NV)
                        nc.vector.tensor_tensor(l14, p4[:, 0:4], p4[:, 4:8], mybir.AluOpType.add)
                        l2 = work_pool.tile([128, 2 * NV * ncol], mid, name=f"l2_{j}", tag="l2")
                        l24 = l2.rearrange("p (k v x) -> p k v x", k=2, v=NV)
                        nc.vector.tensor_tensor(l24, l14[:, 0:2], l14[:, 2:4], mybir.AluOpType.add)
                qk = pipe_pool.tile([128, NV * ncol], mid, name=f"qk{j}", tag="qk")
                qk4 = qk.rearrange("p (v x) -> p v x", v=NV).unsqueeze(1)
                nc.vector.tensor_tensor(qk4, l24[:, 0:1], l24[:, 1:2], mybir.AluOpType.add)

                # e = exp(qk / sqrt(NK)); softmax denominators over v
                e = pipe_pool.tile([128, NV * ncol], f32, name=f"e{j}", tag="e")
                nc.scalar.activation(e, qk, mybir.ActivationFunctionType.Exp, scale=_SCALE)
                t1 = pipe_pool.tile([128, 4 * ncol], f32, name=f"t1_{j}", tag="t1", bufs=1)
                nc.vector.tensor_tensor(t1, e[:, 0:4 * ncol], e[:, 4 * ncol:], mybir.AluOpType.add)
                # t2 / s / r share one scratch tile (padding control)
                sc = pipe_pool.tile([128, 4 * ncol], f32, name=f"sc{j}", tag="sc", bufs=1)
                t2 = sc[:, 0:2 * ncol]
                nc.vector.tensor_tensor(t2, t1[:, 0:2 * ncol], t1[:, 2 * ncol:], mybir.AluOpType.add)
                s = sc[:, 2 * ncol:3 * ncol]
                nc.vector.tensor_tensor(s, t2[:, 0:ncol], t2[:, ncol:], mybir.AluOpType.add)
                r = sc[:, 3 * ncol:4 * ncol]
                if recip_on_act:
                    # r = exp(-ln s): needs two ACT table sets (thrash) but
                    # stays off the DVE critical path
                    ls = sc[:, 0:ncol]
                    nc.scalar.activation(ls, s, mybir.ActivationFunctionType.Ln)
                    nc.scalar.activation(r, ls, mybir.ActivationFunctionType.Exp, scale=-1.0)
                else:
                    nc.vector.reciprocal(r, s)

                # out[v] = e[v] * V[v] * r  (both multiplies in-place on e; DVE
                # executes them after the t-tree reads of e).  The stride-0
                # broadcast operand goes in in0 — a stride-0 in1 runs at half
                # rate on DVE.
                e3 = e.rearrange("p (v x) -> p v x", v=NV)
                nc.vector.tensor_tensor(e3, e3, v_t.rearrange("p (v x) -> p v x", v=NV),
                                        mybir.AluOpType.mult)
                r_b = r.unsqueeze(1).broadcast_to((128, NV, ncol))
                nc.vector.tensor_tensor(e3, r_b, e3, mybir.AluOpType.mult)
                # one output DMA per chunk on the scalar HWDGE ring (deferred)
                pending_out.append(dict(
                    out=y[0:NV, off:off + npix].rearrange("c (p x) -> p c x", p=128),
                    in_=e.rearrange("p (c x) -> p c x", c=NV),
                ))
                off += npix
            flush_out()
    nc.compile()
    return nc


_NC_CACHE = {}

# default build configuration used by kernel()
BUILD_CFG = {"recip_on_act": False, "bf16_tree": False}


def _get_nc(**cfg):
    cfg = {**BUILD_CFG, **cfg}
    key = tuple(sorted(
        (k, tuple(v) if isinstance(v, list) else v) for k, v in cfg.items()
    ))
    if key not in _NC_CACHE:
        _NC_CACHE[key] = build_nc(**cfg)
    return _NC_CACHE[key]


def make_in_maps(inp):
    in_maps = []
    for core in range(N_CORES):
        b, half = core // 2, core % 2
        shard = np.ascontiguousarray(
            inp[b, :, half * ROWS:(half + 1) * ROWS, :], dtype=np.float32
        ).reshape(C, PIX)
        in_maps.append({"x": shard})
    return in_maps


def assemble_out(results):
    out = np.empty((B, NV, H, W), np.float32)
    for core in range(N_CORES):
        b, half = core // 2, core % 2
        out[b, :, half * ROWS:(half + 1) * ROWS, :] = (
            results[core]["y"].reshape(NV, ROWS, W)
        )
    return out


def run_spmd(inp, trace=False, build_cfg=None, **kwargs):
    """Run the SPMD kernel on 8 cores; returns (full_output, BassKernelResults)."""
    _ensure_path()
    from concourse.bass_utils import run_bass_kernel_spmd

    inp = np.asarray(inp)
    assert inp.shape == (B, C, H, W), inp.shape
    nc = _get_nc(**(build_cfg or {}))
    res = run_bass_kernel_spmd(
        nc, make_in_maps(inp), list(range(N_CORES)), trace=trace, **kwargs
    )
    return assemble_out(res.results), res


def kernel(inp):
    out, _ = run_spmd(inp, trace=False)
    return out


# revision 25
# speedup vs baseline: 1.0050x; 1.0050x over previous
"""Trainium2 Bass kernel for per-pixel dot-product attention.

Reference op (per pixel, over C=80 channels split q/k/v = 8/64/8):
    qk[v] = sum_k q[k] * K[k, v] / sqrt(8)
    attn  = softmax(qk over v)
    out[v] = attn[v] * V[v]

Strategy: pure data-parallel over 8 NeuronCores — core i handles batch
i//2, H-rows half (i%2).  Per core all compute is elementwise on
(128, ncol) pixel grids; the 80 channels live as column-blocks of big
SBUF tiles so the whole per-pixel matvec+softmax is ~11 wide vector ops
per chunk (no PSUM / TensorE / transposes).  DVE does the multiplies &
small adds, GPSIMD the big add-tree levels, ScalarE the exp.
"""

import numpy as np

NK = 8
NV = 8
C = NK + NK * NV + NV  # 80
B, H, W = 4, 512, 512
N_CORES = 8
ROWS = H // 2            # rows per core
PIX = ROWS * W           # pixels per core (131072)
NCHUNK = 8               # chunks per core
_SCALE = 1.0 / float(np.sqrt(NK))


def _ensure_path():
    import sys
    p = "/opt/trn_rl_repo"
    if p not in sys.path:
        sys.path.insert(0, p)


def build_nc(pix=PIX, nchunk=NCHUNK, recip_on_act=False, bf16_tree=False,
             k_splits=2, inplace_tree=False, lean_bufs=False, conv_k=False,
             chunk_cols=None, split_rings=False, stage_k=False):
    """Build the per-core Bass program for a (80, pix) f32 shard.

    All tensor_tensor work runs on DVE (GPSIMD shares an SBUF port with DVE
    and the two engines serialize, so Pool offload is a net loss).  K streams
    in on the sync HWDGE ring in `k_splits` pieces (compute starts after the
    first piece); q/v loads and the output go on the scalar ring.  With
    `inplace_tree` the l1/l2 add-tree levels write back into the prod tile
    (strictly trailing writes, single-engine serial) to fit ncol=256 in SBUF.
    """
    _ensure_path()
    import concourse.tile as tile
    from concourse import bacc, mybir

    f32 = mybir.dt.float32
    mid = mybir.dt.bfloat16 if bf16_tree else f32
    if chunk_cols is None:
        npix = pix // nchunk
        assert npix % 128 == 0
        chunk_cols = [npix // 128] * nchunk
    assert sum(chunk_cols) * 128 == pix

    nc = bacc.Bacc("TRN2", target_bir_lowering=False, debug=False)
    x = nc.dram_tensor("x", [C, pix], f32, kind="ExternalInput")
    y = nc.dram_tensor("y", [NV, pix], f32, kind="ExternalOutput")

    qv_bufs = 1 if lean_bufs else 2
    pipe_bufs = 1 if lean_bufs else 2
    # deferred output DMAs: emit chunk j's store after chunk j+1's input
    # triggers so it never head-of-line-blocks loads on its ring
    pending_out = []

    def flush_out():
        for args in pending_out:
            nc.scalar.dma_start(**args)
        pending_out.clear()

    with tile.TileContext(nc) as tc:
        with (
            tc.tile_pool(name="inp", bufs=2) as in_pool,
            tc.tile_pool(name="work", bufs=1) as work_pool,
            tc.tile_pool(name="pipe", bufs=pipe_bufs) as pipe_pool,
        ):
            off = 0
            for j, ncol in enumerate(chunk_cols):
                npix = 128 * ncol
                q_t = in_pool.tile([128, NK * ncol], f32, name=f"q{j}", tag="q", bufs=qv_bufs)
                v_t = in_pool.tile([128, NV * ncol], f32, name=f"v{j}", tag="v", bufs=qv_bufs)

                q_ring = nc.scalar if split_rings else nc.sync
                q_ring.dma_start(
                    out=q_t.rearrange("p (c x) -> p c x", c=NK),
                    in_=x[0:NK, off:off + npix].rearrange("c (p x) -> p c x", p=128),
                )
                # K channel layout is k-major (channel NK + k*NV + v), matching
                # the prod block order; split loads so prod can start early
                sp_ch = NK * NV // k_splits
                if conv_k:  # match the B-half-first compute order
                    dma_order = [h for h in range(k_splits) if h * sp_ch >= NK * NV // 2] + \
                                [h for h in range(k_splits) if h * sp_ch < NK * NV // 2]
                else:
                    dma_order = list(range(k_splits))
                k_stages = {}
                if not stage_k:
                    k_t = in_pool.tile([128, NK * NV * ncol], f32, name=f"k{j}", tag="k")
                    k4 = k_t.rearrange("p (k v x) -> p k v x", k=NK, v=NV)
                for h in dma_order:
                    if stage_k:
                        # small f32 staging ring; ScalarE converts each piece
                        # into the bf16 prod tiles right after it lands
                        kst = in_pool.tile([128, sp_ch * ncol], f32,
                                           name=f"kst{j}_{h}", tag="kst", bufs=2)
                        k_stages[h] = kst
                        dst = kst.rearrange("p (c x) -> p c x", c=sp_ch)
                    else:
                        dst = k_t.rearrange("p (c x) -> p c x", c=NK * NV)[
                            :, h * sp_ch:(h + 1) * sp_ch]
                    on_sync = (h * sp_ch >= NK * NV // 2) or not split_rings
                    ring = nc.sync if on_sync else nc.scalar
                    ring.dma_start(
                        out=dst,
                        in_=x[NK + h * sp_ch:NK + (h + 1) * sp_ch, off:off + npix]
                        .rearrange("c (p x) -> p c x", p=128),
                    )
                nc.sync.dma_start(
                    out=v_t.rearrange("p (c x) -> p c x", c=NV),
                    in_=x[NK + NK * NV:C, off:off + npix]
                    .rearrange("c (p x) -> p c x", p=128),
                )
                # previous chunk's output, behind this chunk's input triggers
                flush_out()

                # prod[k,v] = q[k] * K[k,v]   (one broadcast multiply per K piece)
                sp_k = NK // k_splits
                if conv_k:
                    # ScalarE downcasts K into the prod tiles and Q into a small
                    # bf16 tile; DVE then multiplies in-place at bf16 2x rate.
                    # prod is split into two half-tiles: B frees after l1, so
                    # the next chunk's conversions overlap this chunk's tail.
                    assert bf16_tree and inplace_tree
                    half = NK // 2
                    prodA = work_pool.tile([128, half * NV * ncol], mid,
                                           name=f"prodA{j}", tag="prodA")
                    prodB = work_pool.tile([128, half * NV * ncol], mid,
                                           name=f"prodB{j}", tag="prodB")
                    p4A = prodA.rearrange("p (k v x) -> p k v x", k=half, v=NV)
                    p4B = prodB.rearrange("p (k v x) -> p k v x", k=half, v=NV)
                    q_bf = work_pool.tile([128, NK * ncol], mybir.dt.bfloat16,
                                          name=f"qbf{j}", tag="qbf")
                    nc.scalar.activation(q_bf, q_t, mybir.ActivationFunctionType.Copy)
                    q_b = (
                        q_bf.rearrange("p (k x) -> p k x", k=NK)
                        .unsqueeze(2)
                        .broadcast_to((128, NK, NV, ncol))
                    )

                    def pslice(kl, kh):  # view of prod blocks [kl, kh)
                        if kh <= half:
                            return p4A[:, kl:kh]
                        assert kl >= half
                        return p4B[:, kl - half:kh - half]

                    # emit B-half first: its tile frees earliest (after l1)
                    order = [h for h in range(k_splits) if h * sp_k >= half] + \
                            [h for h in range(k_splits) if h * sp_k < half]
                    for h in order:
                        kl, kh = h * sp_k, (h + 1) * sp_k
                        pv = pslice(kl, kh)
                        if stage_k:
                            src = k_stages[h].rearrange("p (k v x) -> p k v x",
                                                        k=sp_k, v=NV)
                        else:
                            src = k4[:, kl:kh]
                        nc.scalar.activation(pv, src,
                                             mybir.ActivationFunctionType.Copy)
                        nc.vector.tensor_tensor(
                            pv, q_b[:, kl:kh], pv, mybir.AluOpType.mult
                        )
                    # tree: l1 = A + B -> A; l2, qk within A
                    nc.vector.tensor_tensor(p4A, p4A, p4B, mybir.AluOpType.add)
                    l24 = p4A[:, 0:2]
                    nc.vector.tensor_tensor(l24, p4A[:, 0:2], p4A[:, 2:4], mybir.AluOpType.add)
                else:
                    prod = work_pool.tile([128, NK * NV * ncol], mid, name=f"prod{j}", tag="prod")
                    p4 = prod.rearrange("p (k v x) -> p k v x", k=NK, v=NV)
                    q_b = (
                        q_t.rearrange("p (k x) -> p k x", k=NK)
                        .unsqueeze(2)
                        .broadcast_to((128, NK, NV, ncol))
                    )
                    for h in range(k_splits):
                        kl, kh = h * sp_k, (h + 1) * sp_k
                        nc.vector.tensor_tensor(
                            p4[:, kl:kh], q_b[:, kl:kh], k4[:, kl:kh], mybir.AluOpType.mult
                        )

                    # sum over k (outer block index): 3-level pairwise tree (all DVE)
                    if inplace_tree:
                        # l1 -> prod[k 0:4], l2 -> prod[k 0:2]: strictly in-place
                        # (out == in0), serial on DVE
                        l14 = p4[:, 0:4]
                        nc.vector.tensor_tensor(l14, p4[:, 0:4], p4[:, 4:8], mybir.AluOpType.add)
                        l24 = p4[:, 0:2]
                        nc.vector.tensor_tensor(l24, l14[:, 0:2], l14[:, 2:4], mybir.AluOpType.add)
                    else:
                        l1 = work_pool.tile([128, 4 * NV * ncol], mid, name=f"l1_{j}", tag="l1")
                        l14 = l1.rearrange("p (k v x) -> p k v x", k=4, v=NV)
                        nc.vector.tensor_tensor(l14, p4[:, 0:4], p4[:, 4:8], mybir.AluOpType.add)
                        l2 = work_pool.tile([128, 2 * NV * ncol], mid, name=f"l2_{j}", tag="l2")
                        l24 = l2.rearrange("p (k v x) -> p k v x", k=2, v=NV)
                        nc.vector.tensor_tensor(l24, l14[:, 0:2], l14[:, 2:4], mybir.AluOpType.add)
                qk = pipe_pool.tile([128, NV * ncol], mid, name=f"qk{j}", tag="qk")
                qk4 = qk.rearrange("p (v x) -> p v x", v=NV).unsqueeze(1)
                nc.vector.tensor_tensor(qk4, l24[:, 0:1], l24[:, 1:2], mybir.AluOpType.add)

                # e = exp(qk / sqrt(NK)); softmax denominators over v
                e = pipe_pool.tile([128, NV * ncol], f32, name=f"e{j}", tag="e")
                nc.scalar.activation(e, qk, mybir.ActivationFunctionType.Exp, scale=_SCALE)
                t1 = pipe_pool.tile([128, 4 * ncol], f32, name=f"t1_{j}", tag="t1", bufs=1)
                nc.vector.tensor_tensor(t1, e[:, 0:4 * ncol], e[:, 4 * ncol:], mybir.AluOpType.add)
                # t2 / s / r share one scratch tile (padding control)
                sc = pipe_pool.tile([128, 4 * ncol], f32, name=f"sc{j}", tag="sc", bufs=1)
                t2 = sc[:, 0:2 * ncol]
                nc.vector.tensor_tensor(t2, t1[:, 0:2 * ncol], t1[:, 2 * ncol:], mybir.AluOpType.add)
                s = sc[:, 2 * ncol:3 * ncol]
                nc.vector.tensor_tensor(s, t2[:, 0:ncol], t2[:, ncol:], mybir.AluOpType.add)
                r = sc[:, 3 * ncol:4 * ncol]
                if recip_on_act:
                    # r = exp(-ln s): needs two ACT table sets (thrash) but
                    # stays off the DVE critical path
                    ls = sc[:, 0:ncol]
                    nc.scalar.activation(ls, s, mybir.ActivationFunctionType.Ln)
                    nc.scalar.activation(r, ls, mybir.ActivationFunctionType.Exp, scale=-1.0)
                else:
                    nc.vector.reciprocal(r, s)

                # out[v] = e[v] * V[v] * r  (both multiplies in-place on e; DVE
                # executes them after the t-tree reads of e).  The stride-0
                # broadcast operand goes in in0 — a stride-0 in1 runs at half
                # rate on DVE.
                e3 = e.rearrange("p (v x) -> p v x", v=NV)
                nc.vector.tensor_tensor(e3, e3, v_t.rearrange("p (v x) -> p v x", v=NV),
                                        mybir.AluOpType.mult)
                r_b = r.unsqueeze(1).broadcast_to((128, NV, ncol))
                nc.vector.tensor_tensor(e3, r_b, e3, mybir.AluOpType.mult)
                # one output DMA per chunk on the scalar HWDGE ring (deferred)
                pending_out.append(dict(
                    out=y[0:NV, off:off + npix].rearrange("c (p x) -> p c x", p=128),
                    in_=e.rearrange("p (c x) -> p c x", c=NV),
                ))
                off += npix
            flush_out()
    nc.compile()
    return nc


_NC_CACHE = {}

# default build configuration used by kernel()
BUILD_CFG = {"recip_on_act": False, "bf16_tree": False}


def _get_nc(**cfg):
    cfg = {**BUILD_CFG, **cfg}
    key = tuple(sorted(
        (k, tuple(v) if isinstance(v, list) else v) for k, v in cfg.items()
    ))
    if key not in _NC_CACHE:
        _NC_CACHE[key] = build_nc(**cfg)
    return _NC_CACHE[key]


def make_in_maps(inp):
    in_maps = []
    for core in range(N_CORES):
        b, half = core // 2, core % 2
        shard = np.ascontiguousarray(
            inp[b, :, half * ROWS:(half + 1) * ROWS, :], dtype=np.float32
        ).reshape(C, PIX)
        in_maps.append({"x": shard})
    return in_maps


def assemble_out(results):
    out = np.empty((B, NV, H, W), np.float32)
    for core in range(N_CORES):
        b, half = core // 2, core % 2
        out[b, :, half * ROWS:(half + 1) * ROWS, :] = (
            results[core]["y"].reshape(NV, ROWS, W)
        )
    return out


def run_spmd(inp, trace=False, build_cfg=None, **kwargs):
    """Run the SPMD kernel on 8 cores; returns (full_output, BassKernelResults)."""
    _ensure_path()
    from concourse.bass_utils import run_bass_kernel_spmd

    inp = np.asarray(inp)
    assert inp.shape == (B, C, H, W), inp.shape
    nc = _get_nc(**(build_cfg or {}))
    res = run_bass_kernel_spmd(
        nc, make_in_maps(inp), list(range(N_CORES)), trace=trace, **kwargs
    )
    return assemble_out(res.results), res


def kernel(inp):
    out, _ = run_spmd(inp, trace=False)
    return out


# revision 27
# speedup vs baseline: 1.1165x; 1.1110x over previous
"""Trainium2 Bass kernel for per-pixel dot-product attention.

Reference op (per pixel, over C=80 channels split q/k/v = 8/64/8):
    qk[v] = sum_k q[k] * K[k, v] / sqrt(8)
    attn  = softmax(qk over v)
    out[v] = attn[v] * V[v]

Strategy: pure data-parallel over 8 NeuronCores — core i handles batch
i//2, H-rows half (i%2).  Per core all compute is elementwise on
(128, ncol) pixel grids; the 80 channels live as column-blocks of big
SBUF tiles so the whole per-pixel matvec+softmax is ~11 wide vector ops
per chunk (no PSUM / TensorE / transposes).  DVE does the multiplies &
small adds, GPSIMD the big add-tree levels, ScalarE the exp.
"""

import numpy as np

NK = 8
NV = 8
C = NK + NK * NV + NV  # 80
B, H, W = 4, 512, 512
N_CORES = 8
ROWS = H // 2            # rows per core
PIX = ROWS * W           # pixels per core (131072)
NCHUNK = 8               # chunks per core
_SCALE = 1.0 / float(np.sqrt(NK))


def _ensure_path():
    import sys
    p = "/opt/trn_rl_repo"
    if p not in sys.path:
        sys.path.insert(0, p)


def build_nc(pix=PIX, nchunk=NCHUNK, recip_on_act=False, bf16_tree=False,
             k_splits=2, inplace_tree=False, lean_bufs=False, conv_k=False,
             chunk_cols=None, split_rings=False, stage_k=False, direct_b=False):
    """Build the per-core Bass program for a (80, pix) f32 shard.

    All tensor_tensor work runs on DVE (GPSIMD shares an SBUF port with DVE
    and the two engines serialize, so Pool offload is a net loss).  K streams
    in on the sync HWDGE ring in `k_splits` pieces (compute starts after the
    first piece); q/v loads and the output go on the scalar ring.  With
    `inplace_tree` the l1/l2 add-tree levels write back into the prod tile
    (strictly trailing writes, single-engine serial) to fit ncol=256 in SBUF.
    """
    _ensure_path()
    import concourse.tile as tile
    from concourse import bacc, mybir

    f32 = mybir.dt.float32
    mid = mybir.dt.bfloat16 if bf16_tree else f32
    if chunk_cols is None:
        npix = pix // nchunk
        assert npix % 128 == 0
        chunk_cols = [npix // 128] * nchunk
    assert sum(chunk_cols) * 128 == pix

    nc = bacc.Bacc("TRN2", target_bir_lowering=False, debug=False)
    x = nc.dram_tensor("x", [C, pix], f32, kind="ExternalInput")
    y = nc.dram_tensor("y", [NV, pix], f32, kind="ExternalOutput")

    qv_bufs = 1 if lean_bufs else 2
    pipe_bufs = 1 if lean_bufs else 2
    # deferred output DMAs: emit chunk j's store after chunk j+1's input
    # triggers so it never head-of-line-blocks loads on its ring
    pending_out = []

    def flush_out():
        for args in pending_out:
            nc.scalar.dma_start(**args)
        pending_out.clear()

    with tile.TileContext(nc) as tc:
        with (
            tc.tile_pool(name="inp", bufs=2) as in_pool,
            tc.tile_pool(name="work", bufs=1) as work_pool,
            tc.tile_pool(name="pipe", bufs=pipe_bufs) as pipe_pool,
        ):
            off = 0
            for j, ncol in enumerate(chunk_cols):
                npix = 128 * ncol
                q_t = in_pool.tile([128, NK * ncol], f32, name=f"q{j}", tag="q", bufs=qv_bufs)
                v_t = in_pool.tile([128, NV * ncol], f32, name=f"v{j}", tag="v", bufs=qv_bufs)

                q_ring = nc.scalar if split_rings else nc.sync
                q_ring.dma_start(
                    out=q_t.rearrange("p (c x) -> p c x", c=NK),
                    in_=x[0:NK, off:off + npix].rearrange("c (p x) -> p c x", p=128),
                )
                # K channel layout is k-major (channel NK + k*NV + v), matching
                # the prod block order; split loads so prod can start early
                sp_ch = NK * NV // k_splits
                if conv_k:  # match the B-half-first compute order
                    dma_order = [h for h in range(k_splits) if h * sp_ch >= NK * NV // 2] + \
                                [h for h in range(k_splits) if h * sp_ch < NK * NV // 2]
                else:
                    dma_order = list(range(k_splits))
                k_stages = {}
                if not stage_k:
                    k_t = in_pool.tile([128, NK * NV * ncol], f32, name=f"k{j}", tag="k")
                    k4 = k_t.rearrange("p (k v x) -> p k v x", k=NK, v=NV)
                for h in dma_order:
                    if stage_k:
                        # small f32 staging ring; ScalarE converts each piece
                        # into the bf16 prod tiles right after it lands
                        kst = in_pool.tile([128, sp_ch * ncol], f32,
                                           name=f"kst{j}_{h}", tag="kst", bufs=2)
                        k_stages[h] = kst
                        dst = kst.rearrange("p (c x) -> p c x", c=sp_ch)
                    else:
                        dst = k_t.rearrange("p (c x) -> p c x", c=NK * NV)[
                            :, h * sp_ch:(h + 1) * sp_ch]
                    on_sync = (h * sp_ch >= NK * NV // 2) or not split_rings
                    ring = nc.sync if on_sync else nc.scalar
                    ring.dma_start(
                        out=dst,
                        in_=x[NK + h * sp_ch:NK + (h + 1) * sp_ch, off:off + npix]
                        .rearrange("c (p x) -> p c x", p=128),
                    )
                nc.sync.dma_start(
                    out=v_t.rearrange("p (c x) -> p c x", c=NV),
                    in_=x[NK + NK * NV:C, off:off + npix]
                    .rearrange("c (p x) -> p c x", p=128),
                )
                # previous chunk's output, behind this chunk's input triggers
                flush_out()

                # prod[k,v] = q[k] * K[k,v]   (one broadcast multiply per K piece)
                sp_k = NK // k_splits
                if conv_k:
                    # ScalarE downcasts K into the prod tiles and Q into a small
                    # bf16 tile; DVE then multiplies in-place at bf16 2x rate.
                    # prod is split into two half-tiles: B frees after l1, so
                    # the next chunk's conversions overlap this chunk's tail.
                    assert bf16_tree and inplace_tree
                    half = NK // 2
                    prodA = work_pool.tile([128, half * NV * ncol], mid,
                                           name=f"prodA{j}", tag="prodA")
                    prodB = work_pool.tile([128, half * NV * ncol], mid,
                                           name=f"prodB{j}", tag="prodB")
                    p4A = prodA.rearrange("p (k v x) -> p k v x", k=half, v=NV)
                    p4B = prodB.rearrange("p (k v x) -> p k v x", k=half, v=NV)
                    q_bf = work_pool.tile([128, NK * ncol], mybir.dt.bfloat16,
                                          name=f"qbf{j}", tag="qbf")
                    nc.scalar.activation(q_bf, q_t, mybir.ActivationFunctionType.Copy)
                    q_b = (
                        q_bf.rearrange("p (k x) -> p k x", k=NK)
                        .unsqueeze(2)
                        .broadcast_to((128, NK, NV, ncol))
                    )

                    def pslice(kl, kh):  # view of prod blocks [kl, kh)
                        if kh <= half:
                            return p4A[:, kl:kh]
                        assert kl >= half
                        return p4B[:, kl - half:kh - half]

                    # emit B-half first: its tile frees earliest (after l1)
                    order = [h for h in range(k_splits) if h * sp_k >= half] + \
                            [h for h in range(k_splits) if h * sp_k < half]
                    if direct_b:
                        q_bf32 = (
                            q_t.rearrange("p (k x) -> p k x", k=NK)
                            .unsqueeze(2)
                            .broadcast_to((128, NK, NV, ncol))
                        )
                    for h in order:
                        kl, kh = h * sp_k, (h + 1) * sp_k
                        pv = pslice(kl, kh)
                        if stage_k:
                            src = k_stages[h].rearrange("p (k v x) -> p k v x",
                                                        k=sp_k, v=NV)
                        else:
                            src = k4[:, kl:kh]
                        if direct_b and kl >= half:
                            # B half: direct f32 multiply (bf16 out) — no ACT
                            # dependency, so DVE starts as soon as K lands;
                            # ACT meanwhile pre-converts the A half
                            nc.vector.tensor_tensor(
                                pv, q_bf32[:, kl:kh], src, mybir.AluOpType.mult
                            )
                        else:
                            nc.scalar.activation(pv, src,
                                                 mybir.ActivationFunctionType.Copy)
                            nc.vector.tensor_tensor(
                                pv, q_b[:, kl:kh], pv, mybir.AluOpType.mult
                            )
                    # tree: l1 = A + B -> A; l2, qk within A
                    nc.vector.tensor_tensor(p4A, p4A, p4B, mybir.AluOpType.add)
                    l24 = p4A[:, 0:2]
                    nc.vector.tensor_tensor(l24, p4A[:, 0:2], p4A[:, 2:4], mybir.AluOpType.add)
                else:
                    prod = work_pool.tile([128, NK * NV * ncol], mid, name=f"prod{j}", tag="prod")
                    p4 = prod.rearrange("p (k v x) -> p k v x", k=NK, v=NV)
                    q_b = (
                        q_t.rearrange("p (k x) -> p k x", k=NK)
                        .unsqueeze(2)
                        .broadcast_to((128, NK, NV, ncol))
                    )
                    for h in range(k_splits):
                        kl, kh = h * sp_k, (h + 1) * sp_k
                        nc.vector.tensor_tensor(
                            p4[:, kl:kh], q_b[:, kl:kh], k4[:, kl:kh], mybir.AluOpType.mult
                        )

                    # sum over k (outer block index): 3-level pairwise tree (all DVE)
                    if inplace_tree:
                        # l1 -> prod[k 0:4], l2 -> prod[k 0:2]: strictly in-place
                        # (out == in0), serial on DVE
                        l14 = p4[:, 0:4]
                        nc.vector.tensor_tensor(l14, p4[:, 0:4], p4[:, 4:8], mybir.AluOpType.add)
                        l24 = p4[:, 0:2]
                        nc.vector.tensor_tensor(l24, l14[:, 0:2], l14[:, 2:4], mybir.AluOpType.add)
                    else:
                        l1 = work_pool.tile([128, 4 * NV * ncol], mid, name=f"l1_{j}", tag="l1")
                        l14 = l1.rearrange("p (k v x) -> p k v x", k=4, v=NV)
                        nc.vector.tensor_tensor(l14, p4[:, 0:4], p4[:, 4:8], mybir.AluOpType.add)
                        l2 = work_pool.tile([128, 2 * NV * ncol], mid, name=f"l2_{j}", tag="l2")
                        l24 = l2.rearrange("p (k v x) -> p k v x", k=2, v=NV)
                        nc.vector.tensor_tensor(l24, l14[:, 0:2], l14[:, 2:4], mybir.AluOpType.add)
                qk = pipe_pool.tile([128, NV * ncol], mid, name=f"qk{j}", tag="qk")
                qk4 = qk.rearrange("p (v x) -> p v x", v=NV).unsqueeze(1)
                nc.vector.tensor_tensor(qk4, l24[:, 0:1], l24[:, 1:2], mybir.AluOpType.add)

                # e = exp(qk / sqrt(NK)); softmax denominators over v
                e = pipe_pool.tile([128, NV * ncol], f32, name=f"e{j}", tag="e")
                nc.scalar.activation(e, qk, mybir.ActivationFunctionType.Exp, scale=_SCALE)
                t1 = pipe_pool.tile([128, 4 * ncol], f32, name=f"t1_{j}", tag="t1", bufs=1)
                nc.vector.tensor_tensor(t1, e[:, 0:4 * ncol], e[:, 4 * ncol:], mybir.AluOpType.add)
                # t2 / s / r share one scratch tile (padding control)
                sc = pipe_pool.tile([128, 4 * ncol], f32, name=f"sc{j}", tag="sc", bufs=1)
                t2 = sc[:, 0:2 * ncol]
                nc.vector.tensor_tensor(t2, t1[:, 0:2 * ncol], t1[:, 2 * ncol:], mybir.AluOpType.add)
                s = sc[:, 2 * ncol:3 * ncol]
                nc.vector.tensor_tensor(s, t2[:, 0:ncol], t2[:, ncol:], mybir.AluOpType.add)
                r = sc[:, 3 * ncol:4 * ncol]
                if recip_on_act:
                    # r = exp(-ln s): needs two ACT table sets (thrash) but
                    # stays off the DVE critical path
                    ls = sc[:, 0:ncol]
                    nc.scalar.activation(ls, s, mybir.ActivationFunctionType.Ln)
                    nc.scalar.activation(r, ls, mybir.ActivationFunctionType.Exp, scale=-1.0)
                else:
                    nc.vector.reciprocal(r, s)

                # out[v] = e[v] * V[v] * r  (both multiplies in-place on e; DVE
                # executes them after the t-tree reads of e).  The stride-0
                # broadcast operand goes in in0 — a stride-0 in1 runs at half
                # rate on DVE.
                e3 = e.rearrange("p (v x) -> p v x", v=NV)
                nc.vector.tensor_tensor(e3, e3, v_t.rearrange("p (v x) -> p v x", v=NV),
                                        mybir.AluOpType.mult)
                r_b = r.unsqueeze(1).broadcast_to((128, NV, ncol))
                nc.vector.tensor_tensor(e3, r_b, e3, mybir.AluOpType.mult)
                # one output DMA per chunk on the scalar HWDGE ring (deferred)
                pending_out.append(dict(
                    out=y[0:NV, off:off + npix].rearrange("c (p x) -> p c x", p=128),
                    in_=e.rearrange("p (c x) -> p c x", c=NV),
                ))
                off += npix
            flush_out()
    nc.compile()
    return nc


_NC_CACHE = {}

# default build configuration used by kernel()
BUILD_CFG = {"recip_on_act": False, "bf16_tree": False}


def _get_nc(**cfg):
    cfg = {**BUILD_CFG, **cfg}
    key = tuple(sorted(
        (k, tuple(v) if isinstance(v, list) else v) for k, v in cfg.items()
    ))
    if key not in _NC_CACHE:
        _NC_CACHE[key] = build_nc(**cfg)
    return _NC_CACHE[key]


def make_in_maps(inp):
    in_maps = []
    for core in range(N_CORES):
        b, half = core // 2, core % 2
        shard = np.ascontiguousarray(
            inp[b, :, half * ROWS:(half + 1) * ROWS, :], dtype=np.float32
        ).reshape(C, PIX)
        in_maps.append({"x": shard})
    return in_maps


def assemble_out(results):
    out = np.empty((B, NV, H, W), np.float32)
    for core in range(N_CORES):
        b, half = core // 2, core % 2
        out[b, :, half * ROWS:(half + 1) * ROWS, :] = (
            results[core]["y"].reshape(NV, ROWS, W)
        )
    return out


def run_spmd(inp, trace=False, build_cfg=None, **kwargs):
    """Run the SPMD kernel on 8 cores; returns (full_output, BassKernelResults)."""
    _ensure_path()
    from concourse.bass_utils import run_bass_kernel_spmd

    inp = np.asarray(inp)
    assert inp.shape == (B, C, H, W), inp.shape
    nc = _get_nc(**(build_cfg or {}))
    res = run_bass_kernel_spmd(
        nc, make_in_maps(inp), list(range(N_CORES)), trace=trace, **kwargs
    )
    return assemble_out(res.results), res


def kernel(inp):
    out, _ = run_spmd(inp, trace=False)
    return out


# revision 29
# speedup vs baseline: 1.1477x; 1.0279x over previous
"""Trainium2 Bass kernel for per-pixel dot-product attention.

Reference op (per pixel, over C=80 channels split q/k/v = 8/64/8):
    qk[v] = sum_k q[k] * K[k, v] / sqrt(8)
    attn  = softmax(qk over v)
    out[v] = attn[v] * V[v]

Strategy: pure data-parallel over 8 NeuronCores — core i handles batch
i//2, H-rows half (i%2).  Per core all compute is elementwise on
(128, ncol) pixel grids; the 80 channels live as column-blocks of big
SBUF tiles so the whole per-pixel matvec+softmax is ~11 wide vector ops
per chunk (no PSUM / TensorE / transposes).  DVE does the multiplies &
small adds, GPSIMD the big add-tree levels, ScalarE the exp.
"""

import numpy as np

NK = 8
NV = 8
C = NK + NK * NV + NV  # 80
B, H, W = 4, 512, 512
N_CORES = 8
ROWS = H // 2            # rows per core
PIX = ROWS * W           # pixels per core (131072)
NCHUNK = 8               # chunks per core
_SCALE = 1.0 / float(np.sqrt(NK))


def _ensure_path():
    import sys
    p = "/opt/trn_rl_repo"
    if p not in sys.path:
        sys.path.insert(0, p)


def build_nc(pix=PIX, nchunk=NCHUNK, recip_on_act=False, bf16_tree=False,
             k_splits=2, inplace_tree=False, lean_bufs=False, conv_k=False,
             chunk_cols=None, split_rings=False, stage_k=False, direct_b=False):
    """Build the per-core Bass program for a (80, pix) f32 shard.

    All tensor_tensor work runs on DVE (GPSIMD shares an SBUF port with DVE
    and the two engines serialize, so Pool offload is a net loss).  K streams
    in on the sync HWDGE ring in `k_splits` pieces (compute starts after the
    first piece); q/v loads and the output go on the scalar ring.  With
    `inplace_tree` the l1/l2 add-tree levels write back into the prod tile
    (strictly trailing writes, single-engine serial) to fit ncol=256 in SBUF.
    """
    _ensure_path()
    import concourse.tile as tile
    from concourse import bacc, mybir

    f32 = mybir.dt.float32
    mid = mybir.dt.bfloat16 if bf16_tree else f32
    if chunk_cols is None:
        npix = pix // nchunk
        assert npix % 128 == 0
        chunk_cols = [npix // 128] * nchunk
    assert sum(chunk_cols) * 128 == pix

    nc = bacc.Bacc("TRN2", target_bir_lowering=False, debug=False)
    x = nc.dram_tensor("x", [C, pix], f32, kind="ExternalInput")
    y = nc.dram_tensor("y", [NV, pix], f32, kind="ExternalOutput")

    qv_bufs = 1 if lean_bufs else 2
    pipe_bufs = 1 if lean_bufs else 2
    # deferred output DMAs: emit chunk j's store after chunk j+1's input
    # triggers so it never head-of-line-blocks loads on its ring
    pending_out = []

    def flush_out():
        for args in pending_out:
            nc.scalar.dma_start(**args)
        pending_out.clear()

    with tile.TileContext(nc) as tc:
        with (
            tc.tile_pool(name="inp", bufs=2) as in_pool,
            tc.tile_pool(name="work", bufs=1) as work_pool,
            tc.tile_pool(name="pipe", bufs=pipe_bufs) as pipe_pool,
        ):
            off = 0
            for j, ncol in enumerate(chunk_cols):
                npix = 128 * ncol
                q_t = in_pool.tile([128, NK * ncol], f32, name=f"q{j}", tag="q", bufs=qv_bufs)
                v_t = in_pool.tile([128, NV * ncol], f32, name=f"v{j}", tag="v", bufs=qv_bufs)

                q_ring = nc.scalar if split_rings else nc.sync
                q_ring.dma_start(
                    out=q_t.rearrange("p (c x) -> p c x", c=NK),
                    in_=x[0:NK, off:off + npix].rearrange("c (p x) -> p c x", p=128),
                )
                # K channel layout is k-major (channel NK + k*NV + v), matching
                # the prod block order; split loads so prod can start early
                sp_ch = NK * NV // k_splits
                if conv_k:  # match the B-half-first compute order
                    dma_order = [h for h in range(k_splits) if h * sp_ch >= NK * NV // 2] + \
                                [h for h in range(k_splits) if h * sp_ch < NK * NV // 2]
                else:
                    dma_order = list(range(k_splits))
                k_stages = {}
                if not stage_k:
                    k_t = in_pool.tile([128, NK * NV * ncol], f32, name=f"k{j}", tag="k")
                    k4 = k_t.rearrange("p (k v x) -> p k v x", k=NK, v=NV)
                for h in dma_order:
                    if stage_k:
                        # small f32 staging ring; ScalarE converts each piece
                        # into the bf16 prod tiles right after it lands
                        kst = in_pool.tile([128, sp_ch * ncol], f32,
                                           name=f"kst{j}_{h}", tag="kst", bufs=2)
                        k_stages[h] = kst
                        dst = kst.rearrange("p (c x) -> p c x", c=sp_ch)
                    else:
                        dst = k_t.rearrange("p (c x) -> p c x", c=NK * NV)[
                            :, h * sp_ch:(h + 1) * sp_ch]
                    on_sync = (h * sp_ch >= NK * NV // 2) or not split_rings
                    ring = nc.sync if on_sync else nc.scalar
                    ring.dma_start(
                        out=dst,
                        in_=x[NK + h * sp_ch:NK + (h + 1) * sp_ch, off:off + npix]
                        .rearrange("c (p x) -> p c x", p=128),
                    )
                nc.sync.dma_start(
                    out=v_t.rearrange("p (c x) -> p c x", c=NV),
                    in_=x[NK + NK * NV:C, off:off + npix]
                    .rearrange("c (p x) -> p c x", p=128),
                )
                # previous chunk's output, behind this chunk's input triggers
                flush_out()

                # prod[k,v] = q[k] * K[k,v]   (one broadcast multiply per K piece)
                sp_k = NK // k_splits
                if conv_k:
                    # ScalarE downcasts K into the prod tiles and Q into a small
                    # bf16 tile; DVE then multiplies in-place at bf16 2x rate.
                    # prod is split into two half-tiles: B frees after l1, so
                    # the next chunk's conversions overlap this chunk's tail.
                    assert bf16_tree and inplace_tree
                    half = NK // 2
                    prodA = work_pool.tile([128, half * NV * ncol], mid,
                                           name=f"prodA{j}", tag="prodA")
                    prodB = work_pool.tile([128, half * NV * ncol], mid,
                                           name=f"prodB{j}", tag="prodB")
                    p4A = prodA.rearrange("p (k v x) -> p k v x", k=half, v=NV)
                    p4B = prodB.rearrange("p (k v x) -> p k v x", k=half, v=NV)
                    q_bf = work_pool.tile([128, NK * ncol], mybir.dt.bfloat16,
                                          name=f"qbf{j}", tag="qbf")
                    nc.scalar.activation(q_bf, q_t, mybir.ActivationFunctionType.Copy)
                    q_b = (
                        q_bf.rearrange("p (k x) -> p k x", k=NK)
                        .unsqueeze(2)
                        .broadcast_to((128, NK, NV, ncol))
                    )

                    def pslice(kl, kh):  # view of prod blocks [kl, kh)
                        if kh <= half:
                            return p4A[:, kl:kh]
                        assert kl >= half
                        return p4B[:, kl - half:kh - half]

                    # emit B-half first: its tile frees earliest (after l1)
                    order = [h for h in range(k_splits) if h * sp_k >= half] + \
                            [h for h in range(k_splits) if h * sp_k < half]
                    if direct_b:
                        q_bf32 = (
                            q_t.rearrange("p (k x) -> p k x", k=NK)
                            .unsqueeze(2)
                            .broadcast_to((128, NK, NV, ncol))
                        )
                    for h in order:
                        kl, kh = h * sp_k, (h + 1) * sp_k
                        pv = pslice(kl, kh)
                        if stage_k:
                            src = k_stages[h].rearrange("p (k v x) -> p k v x",
                                                        k=sp_k, v=NV)
                        else:
                            src = k4[:, kl:kh]
                        if direct_b and kl >= half:
                            # B half: direct f32 multiply (bf16 out) — no ACT
                            # dependency, so DVE starts as soon as K lands;
                            # ACT meanwhile pre-converts the A half
                            nc.vector.tensor_tensor(
                                pv, q_bf32[:, kl:kh], src, mybir.AluOpType.mult
                            )
                        else:
                            nc.scalar.activation(pv, src,
                                                 mybir.ActivationFunctionType.Copy)
                            nc.vector.tensor_tensor(
                                pv, q_b[:, kl:kh], pv, mybir.AluOpType.mult
                            )
                    # tree: l1 = A + B -> A; l2, qk within A
                    nc.vector.tensor_tensor(p4A, p4A, p4B, mybir.AluOpType.add)
                    l24 = p4A[:, 0:2]
                    nc.vector.tensor_tensor(l24, p4A[:, 0:2], p4A[:, 2:4], mybir.AluOpType.add)
                else:
                    prod = work_pool.tile([128, NK * NV * ncol], mid, name=f"prod{j}", tag="prod")
                    p4 = prod.rearrange("p (k v x) -> p k v x", k=NK, v=NV)
                    q_b = (
                        q_t.rearrange("p (k x) -> p k x", k=NK)
                        .unsqueeze(2)
                        .broadcast_to((128, NK, NV, ncol))
                    )
                    for h in range(k_splits):
                        kl, kh = h * sp_k, (h + 1) * sp_k
                        nc.vector.tensor_tensor(
                            p4[:, kl:kh], q_b[:, kl:kh], k4[:, kl:kh], mybir.AluOpType.mult
                        )

                    # sum over k (outer block index): 3-level pairwise tree (all DVE)
                    if inplace_tree:
                        # l1 -> prod[k 0:4], l2 -> prod[k 0:2]: strictly in-place
                        # (out == in0), serial on DVE
                        l14 = p4[:, 0:4]
                        nc.vector.tensor_tensor(l14, p4[:, 0:4], p4[:, 4:8], mybir.AluOpType.add)
                        l24 = p4[:, 0:2]
                        nc.vector.tensor_tensor(l24, l14[:, 0:2], l14[:, 2:4], mybir.AluOpType.add)
                    else:
                        l1 = work_pool.tile([128, 4 * NV * ncol], mid, name=f"l1_{j}", tag="l1")
                        l14 = l1.rearrange("p (k v x) -> p k v x", k=4, v=NV)
                        nc.vector.tensor_tensor(l14, p4[:, 0:4], p4[:, 4:8], mybir.AluOpType.add)
                        l2 = work_pool.tile([128, 2 * NV * ncol], mid, name=f"l2_{j}", tag="l2")
                        l24 = l2.rearrange("p (k v x) -> p k v x", k=2, v=NV)
                        nc.vector.tensor_tensor(l24, l14[:, 0:2], l14[:, 2:4], mybir.AluOpType.add)
                # qk shares t1's slot: qk dies at exp, t1 is born after exp
                qk = pipe_pool.tile([128, NV * ncol], mid, name=f"qk{j}", tag="t1", bufs=1)
                qk4 = qk.rearrange("p (v x) -> p v x", v=NV).unsqueeze(1)
                nc.vector.tensor_tensor(qk4, l24[:, 0:1], l24[:, 1:2], mybir.AluOpType.add)

                # e = exp(qk / sqrt(NK)); softmax denominators over v
                e = pipe_pool.tile([128, NV * ncol], f32, name=f"e{j}", tag="e", bufs=2)
                nc.scalar.activation(e, qk, mybir.ActivationFunctionType.Exp, scale=_SCALE)
                t1 = pipe_pool.tile([128, 4 * ncol], f32, name=f"t1_{j}", tag="t1", bufs=1)
                nc.vector.tensor_tensor(t1, e[:, 0:4 * ncol], e[:, 4 * ncol:], mybir.AluOpType.add)
                # t2 / s / r share one scratch tile (padding control)
                sc = pipe_pool.tile([128, 4 * ncol], f32, name=f"sc{j}", tag="sc", bufs=1)
                t2 = sc[:, 0:2 * ncol]
                nc.vector.tensor_tensor(t2, t1[:, 0:2 * ncol], t1[:, 2 * ncol:], mybir.AluOpType.add)
                s = sc[:, 2 * ncol:3 * ncol]
                nc.vector.tensor_tensor(s, t2[:, 0:ncol], t2[:, ncol:], mybir.AluOpType.add)
                r = sc[:, 3 * ncol:4 * ncol]
                if recip_on_act:
                    # r = exp(-ln s): needs two ACT table sets (thrash) but
                    # stays off the DVE critical path
                    ls = sc[:, 0:ncol]
                    nc.scalar.activation(ls, s, mybir.ActivationFunctionType.Ln)
                    nc.scalar.activation(r, ls, mybir.ActivationFunctionType.Exp, scale=-1.0)
                else:
                    nc.vector.reciprocal(r, s)

                # out[v] = e[v] * V[v] * r  (both multiplies in-place on e; DVE
                # executes them after the t-tree reads of e).  The stride-0
                # broadcast operand goes in in0 — a stride-0 in1 runs at half
                # rate on DVE.
                e3 = e.rearrange("p (v x) -> p v x", v=NV)
                nc.vector.tensor_tensor(e3, e3, v_t.rearrange("p (v x) -> p v x", v=NV),
                                        mybir.AluOpType.mult)
                r_b = r.unsqueeze(1).broadcast_to((128, NV, ncol))
                nc.vector.tensor_tensor(e3, r_b, e3, mybir.AluOpType.mult)
                # one output DMA per chunk on the scalar HWDGE ring (deferred)
                pending_out.append(dict(
                    out=y[0:NV, off:off + npix].rearrange("c (p x) -> p c x", p=128),
                    in_=e.rearrange("p (c x) -> p c x", c=NV),
                ))
                off += npix
            flush_out()
    nc.compile()
    return nc


_NC_CACHE = {}

# default build configuration used by kernel()
BUILD_CFG = {"recip_on_act": False, "bf16_tree": False}


def _get_nc(**cfg):
    cfg = {**BUILD_CFG, **cfg}
    key = tuple(sorted(
        (k, tuple(v) if isinstance(v, list) else v) for k, v in cfg.items()
    ))
    if key not in _NC_CACHE:
        _NC_CACHE[key] = build_nc(**cfg)
    return _NC_CACHE[key]


def make_in_maps(inp):
    in_maps = []
    for core in range(N_CORES):
        b, half = core // 2, core % 2
        shard = np.ascontiguousarray(
            inp[b, :, half * ROWS:(half + 1) * ROWS, :], dtype=np.float32
        ).reshape(C, PIX)
        in_maps.append({"x": shard})
    return in_maps


def assemble_out(results):
    out = np.empty((B, NV, H, W), np.float32)
    for core in range(N_CORES):
        b, half = core // 2, core % 2
        out[b, :, half * ROWS:(half + 1) * ROWS, :] = (
            results[core]["y"].reshape(NV, ROWS, W)
        )
    return out


def run_spmd(inp, trace=False, build_cfg=None, **kwargs):
    """Run the SPMD kernel on 8 cores; returns (full_output, BassKernelResults)."""
    _ensure_path()
    from concourse.bass_utils import run_bass_kernel_spmd

    inp = np.asarray(inp)
    assert inp.shape == (B, C, H, W), inp.shape
    nc = _get_nc(**(build_cfg or {}))
    res = run_bass_kernel_spmd(
        nc, make_in_maps(inp), list(range(N_CORES)), trace=trace, **kwargs
    )
    return assemble_out(res.results), res


def kernel(inp):
    out, _ = run_spmd(inp, trace=False)
    return out


# revision 31
# speedup vs baseline: 1.1558x; 1.0070x over previous
"""Trainium2 Bass kernel for per-pixel dot-product attention.

Reference op (per pixel, over C=80 channels split q/k/v = 8/64/8):
    qk[v] = sum_k q[k] * K[k, v] / sqrt(8)
    attn  = softmax(qk over v)
    out[v] = attn[v] * V[v]

Strategy: pure data-parallel over 8 NeuronCores — core i handles batch
i//2, H-rows half (i%2).  Per core all compute is elementwise on
(128, ncol) pixel grids; the 80 channels live as column-blocks of big
SBUF tiles so the whole per-pixel matvec+softmax is ~20 wide vector ops
per chunk (no PSUM / TensorE / transposes / GPSIMD — the Pool engine
shares an SBUF port with DVE and the two serialize).  DVE does the
multiplies and the pairwise add-trees (bf16, 2x mode), ScalarE does the
f32->bf16 downcasts and the exp, and both HWDGE rings stream DMA with
~1KB descriptors.  The kernel is DMA-bound: ~44.5 MB HBM traffic/core
at ~340 GB/s ≈ 130 us, measured 161 us end-to-end.
"""

import numpy as np

NK = 8
NV = 8
C = NK + NK * NV + NV  # 80
B, H, W = 4, 512, 512
N_CORES = 8
ROWS = H // 2            # rows per core
PIX = ROWS * W           # pixels per core (131072)
NCHUNK = 8               # chunks per core
_SCALE = 1.0 / float(np.sqrt(NK))


def _ensure_path():
    import sys
    p = "/opt/trn_rl_repo"
    if p not in sys.path:
        sys.path.insert(0, p)


def build_nc(pix=PIX, nchunk=NCHUNK, recip_on_act=False, bf16_tree=False,
             k_splits=2, inplace_tree=False, lean_bufs=False, conv_k=False,
             chunk_cols=None, split_rings=False, stage_k=False, direct_b=False):
    """Build the per-core Bass program for a (80, pix) f32 shard.

    All tensor_tensor work runs on DVE (GPSIMD shares an SBUF port with DVE
    and the two engines serialize, so Pool offload is a net loss).  K streams
    in on the sync HWDGE ring in `k_splits` pieces (compute starts after the
    first piece); q/v loads and the output go on the scalar ring.  With
    `inplace_tree` the l1/l2 add-tree levels write back into the prod tile
    (strictly trailing writes, single-engine serial) to fit ncol=256 in SBUF.
    """
    _ensure_path()
    import concourse.tile as tile
    from concourse import bacc, mybir

    f32 = mybir.dt.float32
    mid = mybir.dt.bfloat16 if bf16_tree else f32
    if chunk_cols is None:
        npix = pix // nchunk
        assert npix % 128 == 0
        chunk_cols = [npix // 128] * nchunk
    assert sum(chunk_cols) * 128 == pix

    nc = bacc.Bacc("TRN2", target_bir_lowering=False, debug=False)
    x = nc.dram_tensor("x", [C, pix], f32, kind="ExternalInput")
    y = nc.dram_tensor("y", [NV, pix], f32, kind="ExternalOutput")

    qv_bufs = 1 if lean_bufs else 2
    pipe_bufs = 1 if lean_bufs else 2
    # deferred output DMAs: emit chunk j's store after chunk j+1's input
    # triggers so it never head-of-line-blocks loads on its ring
    pending_out = []

    def flush_out():
        for args in pending_out:
            nc.scalar.dma_start(**args)
        pending_out.clear()

    with tile.TileContext(nc) as tc:
        with (
            tc.tile_pool(name="inp", bufs=2) as in_pool,
            tc.tile_pool(name="work", bufs=1) as work_pool,
            tc.tile_pool(name="pipe", bufs=pipe_bufs) as pipe_pool,
        ):
            off = 0
            for j, ncol in enumerate(chunk_cols):
                npix = 128 * ncol
                q_t = in_pool.tile([128, NK * ncol], f32, name=f"q{j}", tag="q", bufs=qv_bufs)
                v_t = in_pool.tile([128, NV * ncol], f32, name=f"v{j}", tag="v", bufs=qv_bufs)

                q_ring = nc.scalar if split_rings else nc.sync
                q_ring.dma_start(
                    out=q_t.rearrange("p (c x) -> p c x", c=NK),
                    in_=x[0:NK, off:off + npix].rearrange("c (p x) -> p c x", p=128),
                )
                # K channel layout is k-major (channel NK + k*NV + v), matching
                # the prod block order; split loads so prod can start early
                sp_ch = NK * NV // k_splits
                if conv_k:  # match the B-half-first compute order
                    dma_order = [h for h in range(k_splits) if h * sp_ch >= NK * NV // 2] + \
                                [h for h in range(k_splits) if h * sp_ch < NK * NV // 2]
                else:
                    dma_order = list(range(k_splits))
                k_stages = {}
                if not stage_k:
                    k_t = in_pool.tile([128, NK * NV * ncol], f32, name=f"k{j}", tag="k")
                    k4 = k_t.rearrange("p (k v x) -> p k v x", k=NK, v=NV)
                for h in dma_order:
                    if stage_k:
                        # small f32 staging ring; ScalarE converts each piece
                        # into the bf16 prod tiles right after it lands
                        kst = in_pool.tile([128, sp_ch * ncol], f32,
                                           name=f"kst{j}_{h}", tag="kst", bufs=2)
                        k_stages[h] = kst
                        dst = kst.rearrange("p (c x) -> p c x", c=sp_ch)
                    else:
                        dst = k_t.rearrange("p (c x) -> p c x", c=NK * NV)[
                            :, h * sp_ch:(h + 1) * sp_ch]
                    on_sync = (h * sp_ch >= NK * NV // 2) or not split_rings
                    ring = nc.sync if on_sync else nc.scalar
                    ring.dma_start(
                        out=dst,
                        in_=x[NK + h * sp_ch:NK + (h + 1) * sp_ch, off:off + npix]
                        .rearrange("c (p x) -> p c x", p=128),
                    )
                nc.sync.dma_start(
                    out=v_t.rearrange("p (c x) -> p c x", c=NV),
                    in_=x[NK + NK * NV:C, off:off + npix]
                    .rearrange("c (p x) -> p c x", p=128),
                )
                # previous chunk's output, behind this chunk's input triggers
                flush_out()

                # prod[k,v] = q[k] * K[k,v]   (one broadcast multiply per K piece)
                sp_k = NK // k_splits
                if conv_k:
                    # ScalarE downcasts K into the prod tiles and Q into a small
                    # bf16 tile; DVE then multiplies in-place at bf16 2x rate.
                    # prod is split into two half-tiles: B frees after l1, so
                    # the next chunk's conversions overlap this chunk's tail.
                    assert bf16_tree and inplace_tree
                    half = NK // 2
                    prodA = work_pool.tile([128, half * NV * ncol], mid,
                                           name=f"prodA{j}", tag="prodA")
                    prodB = work_pool.tile([128, half * NV * ncol], mid,
                                           name=f"prodB{j}", tag="prodB")
                    p4A = prodA.rearrange("p (k v x) -> p k v x", k=half, v=NV)
                    p4B = prodB.rearrange("p (k v x) -> p k v x", k=half, v=NV)
                    q_bf = work_pool.tile([128, NK * ncol], mybir.dt.bfloat16,
                                          name=f"qbf{j}", tag="qbf")
                    nc.scalar.activation(q_bf, q_t, mybir.ActivationFunctionType.Copy)
                    q_b = (
                        q_bf.rearrange("p (k x) -> p k x", k=NK)
                        .unsqueeze(2)
                        .broadcast_to((128, NK, NV, ncol))
                    )

                    def pslice(kl, kh):  # view of prod blocks [kl, kh)
                        if kh <= half:
                            return p4A[:, kl:kh]
                        assert kl >= half
                        return p4B[:, kl - half:kh - half]

                    # emit B-half first: its tile frees earliest (after l1)
                    order = [h for h in range(k_splits) if h * sp_k >= half] + \
                            [h for h in range(k_splits) if h * sp_k < half]
                    if direct_b:
                        q_bf32 = (
                            q_t.rearrange("p (k x) -> p k x", k=NK)
                            .unsqueeze(2)
                            .broadcast_to((128, NK, NV, ncol))
                        )
                    for h in order:
                        kl, kh = h * sp_k, (h + 1) * sp_k
                        pv = pslice(kl, kh)
                        if stage_k:
                            src = k_stages[h].rearrange("p (k v x) -> p k v x",
                                                        k=sp_k, v=NV)
                        else:
                            src = k4[:, kl:kh]
                        if direct_b and kl >= half:
                            # B half: direct f32 multiply (bf16 out) — no ACT
                            # dependency, so DVE starts as soon as K lands;
                            # ACT meanwhile pre-converts the A half
                            nc.vector.tensor_tensor(
                                pv, q_bf32[:, kl:kh], src, mybir.AluOpType.mult
                            )
                        else:
                            nc.scalar.activation(pv, src,
                                                 mybir.ActivationFunctionType.Copy)
                            nc.vector.tensor_tensor(
                                pv, q_b[:, kl:kh], pv, mybir.AluOpType.mult
                            )
                    # tree: l1 = A + B -> A; l2, qk within A
                    nc.vector.tensor_tensor(p4A, p4A, p4B, mybir.AluOpType.add)
                    l24 = p4A[:, 0:2]
                    nc.vector.tensor_tensor(l24, p4A[:, 0:2], p4A[:, 2:4], mybir.AluOpType.add)
                else:
                    prod = work_pool.tile([128, NK * NV * ncol], mid, name=f"prod{j}", tag="prod")
                    p4 = prod.rearrange("p (k v x) -> p k v x", k=NK, v=NV)
                    q_b = (
                        q_t.rearrange("p (k x) -> p k x", k=NK)
                        .unsqueeze(2)
                        .broadcast_to((128, NK, NV, ncol))
                    )
                    for h in range(k_splits):
                        kl, kh = h * sp_k, (h + 1) * sp_k
                        nc.vector.tensor_tensor(
                            p4[:, kl:kh], q_b[:, kl:kh], k4[:, kl:kh], mybir.AluOpType.mult
                        )

                    # sum over k (outer block index): 3-level pairwise tree (all DVE)
                    if inplace_tree:
                        # l1 -> prod[k 0:4], l2 -> prod[k 0:2]: strictly in-place
                        # (out == in0), serial on DVE
                        l14 = p4[:, 0:4]
                        nc.vector.tensor_tensor(l14, p4[:, 0:4], p4[:, 4:8], mybir.AluOpType.add)
                        l24 = p4[:, 0:2]
                        nc.vector.tensor_tensor(l24, l14[:, 0:2], l14[:, 2:4], mybir.AluOpType.add)
                    else:
                        l1 = work_pool.tile([128, 4 * NV * ncol], mid, name=f"l1_{j}", tag="l1")
                        l14 = l1.rearrange("p (k v x) -> p k v x", k=4, v=NV)
                        nc.vector.tensor_tensor(l14, p4[:, 0:4], p4[:, 4:8], mybir.AluOpType.add)
                        l2 = work_pool.tile([128, 2 * NV * ncol], mid, name=f"l2_{j}", tag="l2")
                        l24 = l2.rearrange("p (k v x) -> p k v x", k=2, v=NV)
                        nc.vector.tensor_tensor(l24, l14[:, 0:2], l14[:, 2:4], mybir.AluOpType.add)
                # qk shares t1's slot: qk dies at exp, t1 is born after exp
                qk = pipe_pool.tile([128, NV * ncol], mid, name=f"qk{j}", tag="t1", bufs=1)
                qk4 = qk.rearrange("p (v x) -> p v x", v=NV).unsqueeze(1)
                nc.vector.tensor_tensor(qk4, l24[:, 0:1], l24[:, 1:2], mybir.AluOpType.add)

                # e = exp(qk / sqrt(NK)); softmax denominators over v
                e = pipe_pool.tile([128, NV * ncol], f32, name=f"e{j}", tag="e", bufs=2)
                nc.scalar.activation(e, qk, mybir.ActivationFunctionType.Exp, scale=_SCALE)
                t1 = pipe_pool.tile([128, 4 * ncol], f32, name=f"t1_{j}", tag="t1", bufs=1)
                nc.vector.tensor_tensor(t1, e[:, 0:4 * ncol], e[:, 4 * ncol:], mybir.AluOpType.add)
                # t2 / s / r share one scratch tile (padding control)
                sc = pipe_pool.tile([128, 4 * ncol], f32, name=f"sc{j}", tag="sc", bufs=1)
                t2 = sc[:, 0:2 * ncol]
                nc.vector.tensor_tensor(t2, t1[:, 0:2 * ncol], t1[:, 2 * ncol:], mybir.AluOpType.add)
                s = sc[:, 2 * ncol:3 * ncol]
                nc.vector.tensor_tensor(s, t2[:, 0:ncol], t2[:, ncol:], mybir.AluOpType.add)
                r = sc[:, 3 * ncol:4 * ncol]
                if recip_on_act:
                    # r = exp(-ln s): needs two ACT table sets (thrash) but
                    # stays off the DVE critical path
                    ls = sc[:, 0:ncol]
                    nc.scalar.activation(ls, s, mybir.ActivationFunctionType.Ln)
                    nc.scalar.activation(r, ls, mybir.ActivationFunctionType.Exp, scale=-1.0)
                else:
                    nc.vector.reciprocal(r, s)

                # out[v] = e[v] * V[v] * r  (both multiplies in-place on e; DVE
                # executes them after the t-tree reads of e).  The stride-0
                # broadcast operand goes in in0 — a stride-0 in1 runs at half
                # rate on DVE.
                e3 = e.rearrange("p (v x) -> p v x", v=NV)
                nc.vector.tensor_tensor(e3, e3, v_t.rearrange("p (v x) -> p v x", v=NV),
                                        mybir.AluOpType.mult)
                r_b = r.unsqueeze(1).broadcast_to((128, NV, ncol))
                nc.vector.tensor_tensor(e3, r_b, e3, mybir.AluOpType.mult)
                # one output DMA per chunk on the scalar HWDGE ring (deferred)
                pending_out.append(dict(
                    out=y[0:NV, off:off + npix].rearrange("c (p x) -> p c x", p=128),
                    in_=e.rearrange("p (c x) -> p c x", c=NV),
                ))
                off += npix
            flush_out()
    nc.compile()
    return nc


_NC_CACHE = {}

# default build configuration used by kernel(): bf16 product + add-tree
# (ScalarE converts the A half, DVE multiplies the B half straight from f32),
# ncol=256 chunks with a tapered first/last chunk, in-place tree, outputs
# deferred behind the next chunk's loads.  Measured 161 us/NEFF on trn2
# (8 cores, ~44.5 MB traffic/core ≈ 130 us DMA floor); output rel-l2 vs the
# f32 reference ≈ 3.3e-3 (bf16 rounding of the qk tree).
BUILD_CFG = {
    "recip_on_act": False,
    "bf16_tree": True,
    "k_splits": 8,
    "inplace_tree": True,
    "lean_bufs": True,
    "conv_k": True,
    "direct_b": True,
    "chunk_cols": [128, 256, 256, 256, 128],
}


def _get_nc(**cfg):
    cfg = {**BUILD_CFG, **cfg}
    key = tuple(sorted(
        (k, tuple(v) if isinstance(v, list) else v) for k, v in cfg.items()
    ))
    if key not in _NC_CACHE:
        _NC_CACHE[key] = build_nc(**cfg)
    return _NC_CACHE[key]


def make_in_maps(inp):
    in_maps = []
    for core in range(N_CORES):
        b, half = core // 2, core % 2
        shard = np.ascontiguousarray(
            inp[b, :, half * ROWS:(half + 1) * ROWS, :], dtype=np.float32
        ).reshape(C, PIX)
        in_maps.append({"x": shard})
    return in_maps


def assemble_out(results):
    out = np.empty((B, NV, H, W), np.float32)
    for core in range(N_CORES):
        b, half = core // 2, core % 2
        out[b, :, half * ROWS:(half + 1) * ROWS, :] = (
            results[core]["y"].reshape(NV, ROWS, W)
        )
    return out


def run_spmd(inp, trace=False, build_cfg=None, **kwargs):
    """Run the SPMD kernel on 8 cores; returns (full_output, BassKernelResults)."""
    _ensure_path()
    from concourse.bass_utils import run_bass_kernel_spmd

    inp = np.asarray(inp)
    assert inp.shape == (B, C, H, W), inp.shape
    nc = _get_nc(**(build_cfg or {}))
    res = run_bass_kernel_spmd(
        nc, make_in_maps(inp), list(range(N_CORES)), trace=trace, **kwargs
    )
    return assemble_out(res.results), res


def kernel(inp):
    out, _ = run_spmd(inp, trace=False)
    return out
